# revision 56
# baseline (speedup 1.0000x reference)
"""Multi-head attention (B=4, S=2048, E=1024, H=16, D=64) on 8 TRN2 cores.

Sharding: heads 2c, 2c+1 on core c (Megatron-style column-parallel qkv,
row-parallel out-projection; bf16 partial outputs summed on host).

Per-core dataflow (bf16 operands, f32 PSUM accumulation; ~4e-3 rel err):
  A) qkvT feature-major projection of the (replicated, bf16) xT in 1024-col
     chunks; q is stored as two zero-padded copies (q0: head0 rows live,
     q1: head1 rows live) so score matmuls run K=128 with the full k tile
     stationary (HW-measured: row-tiled K=64 matmuls are ~2x slower).
  B) per (batch, sq-chunk): kt-outer loop: per head, scoresT[sk, sq]
     matmuls, exp on ScalarE (scale=1/8 folded in; |scores| < 3 so no
     max-subtraction), bf16 probs, attnT accumulation per head with [128,
     128] vk stationaries placing h0's denom at PSUM row 0 / attn at rows
     64:127 and h1's attn at rows 0:63 / denom at row 64, so the
     normalization multiplies (reciprocal + gpsimd partition-broadcast +
     DVE multiply) write bf16 ab directly, partition-aligned. at-matmuls
     are flushed two kt behind the exp stream so the PE never stalls the
     score-matmul stream (which feeds ScalarE) on an exp dependency.
  C) row-parallel out-projection of the local 128 features -> bf16 yT
     partial (summed across cores on the host); PSUM->SBUF move + bias on
     ScalarE at group boundaries (where the exp stream idles), stores
     alternate between two DMA queues.

Emission interleaves phase-A chunks of batch b+1 and out-projection parts of
batch b-1 between attention groups of batch b, so the in-order PE always has
dependency-free matmuls to fill exp/normalization stalls. ScalarE exp
(~265us busy/core) is the structural pacer; score+exp runs within ~9us of
that floor.
"""
from contextlib import ExitStack

import numpy as np

import concourse.bass as bass
import concourse.mybir as mybir
import concourse.tile as tile
from concourse import bacc
from concourse.bass_utils import run_bass_kernel_spmd
from concourse.masks import make_identity

B, S, E, H, D = 4, 2048, 1024, 16, 64
NCORES = 8
HPC = H // NCORES        # 2 heads per core
F = HPC * D              # 128 local features
M3 = 3 * F               # 384 local qkv rows
BS = B * S               # 8192
KT_E = E // 128          # 8 contraction tiles for projections
KT_S = S // 128          # 16 sk tiles
AC = 1024                # phase-A chunk width
f32 = mybir.dt.float32
bf16 = mybir.dt.bfloat16
EXP = mybir.ActivationFunctionType.Exp

_prog_cache = {}


def build_program(niter=None, parts="Aao"):
    """niter=None: normal external-I/O program. niter=N: timing variant with
    internal DRAM x/y and the whole body in a device-side For_i loop."""
    key = ("nc", niter, parts)
    if key in _prog_cache:
        return _prog_cache[key]
    nc = bacc.Bacc("TRN2", target_bir_lowering=False)
    if niter is None:
        xT = nc.dram_tensor("xT", [E, BS], bf16, kind="ExternalInput")
        yT = nc.dram_tensor("yT", [E, BS], bf16, kind="ExternalOutput")
    else:
        xT = nc.dram_tensor("xTi", [E, BS], bf16, kind="Internal")
        yT = nc.dram_tensor("yTi", [E, BS], bf16, kind="Internal")
    wq = nc.dram_tensor("wq", [E, M3], bf16, kind="ExternalInput")
    bq = nc.dram_tensor("bq", [128, 3], f32, kind="ExternalInput")
    wo = nc.dram_tensor("wo", [F, E], bf16, kind="ExternalInput")
    bo = nc.dram_tensor("bo", [128, E // 128], f32, kind="ExternalInput")
    if niter is not None:
        tout = nc.dram_tensor("tout", [1, 3], f32, kind="ExternalOutput")

    with tile.TileContext(nc) as tc, ExitStack() as ctx:
        const = ctx.enter_context(tc.tile_pool(name="const", bufs=1))
        xp = ctx.enter_context(tc.tile_pool(name="xp", bufs=2))
        expp = ctx.enter_context(tc.tile_pool(name="expp", bufs=8))
        anp = ctx.enter_context(tc.tile_pool(name="anp", bufs=4))
        asp = ctx.enter_context(tc.tile_pool(name="asp", bufs=2))
        ystp = ctx.enter_context(tc.tile_pool(name="ystp", bufs=8))
        # PSUM: "sc" slots [128,1024] f32 (scores / qkv / out-proj /
        # transposes), "at" slots [65,1024] f32 x2 heads.
        pssc = ctx.enter_context(tc.tile_pool(name="pssc", bufs=2, space="PSUM"))
        psat = ctx.enter_context(tc.tile_pool(name="psat", bufs=2, space="PSUM"))

        wq_sb = const.tile([128, KT_E, M3], bf16)
        nc.gpsimd.dma_start(out=wq_sb, in_=wq.rearrange("(kt p) m -> p kt m", p=128))
        wo_sb = const.tile([F, E], bf16)
        nc.gpsimd.dma_start(out=wo_sb, in_=wo[:, :])
        bq_sb = const.tile([128, 3], f32)
        nc.gpsimd.dma_start(out=bq_sb, in_=bq[:, :])
        bo_sb = const.tile([128, E // 128], f32)
        nc.gpsimd.dma_start(out=bo_sb, in_=bo[:, :])
        id_f32 = const.tile([128, 128], f32)
        make_identity(nc, id_f32)
        id_sb = const.tile([128, 128], bf16)
        nc.vector.tensor_copy(id_sb, id_f32)

        xT_r = xT.rearrange("(kt p) n -> p kt n", p=128)

        # persistent double-buffered qkv storage: [128, 2, S] per tensor,
        # slot b%2. q is written as two zero-padded copies (q0: head0 rows
        # live, head1 rows zero; q1 vice versa) so score matmuls run K=128
        # with the full k tile stationary — HW-measured K=64 row-tiled
        # matmuls and per-matmul weight reloads are slower. vk ones columns
        # and q zero halves are written once.
        q0_st = const.tile([128, 2, S], bf16, name="q0_st")
        q1_st = const.tile([128, 2, S], bf16, name="q1_st")
        k_st = const.tile([128, 2, S], bf16, name="k_st")
        v_st = const.tile([128, 2, S], bf16, name="v_st")
        # vk stationaries are [128, 128] so each head's attnT lands in its
        # own PSUM partition range: h0 = [ones | 0... | v0] -> denom row 0 +
        # attn rows 64:127 (-> ab[64:128]); h1 = [v1 | ones | 0...] -> attn
        # rows 0:63 (-> ab[0:64]) + denom row 64. Normalization multiplies
        # write ab directly on DVE (partition-aligned, no copy DMA); only
        # h1 needs the denominator row-shift DMA (64 -> 0) before the
        # gpsimd partition broadcast. Host reorders W_out columns to match.
        vk_st = const.tile([128, 2, HPC * KT_S, 128], bf16, name="vk_st")
        nc.vector.memset(q0_st[64:128, :, :], 0.0)
        nc.vector.memset(q1_st[0:64, :, :], 0.0)
        nc.vector.memset(vk_st[:, :, 0:KT_S, 0:64], 0.0)
        nc.vector.memset(vk_st[:, :, 0:KT_S, 0:1], 1.0)
        nc.vector.memset(vk_st[:, :, KT_S:2 * KT_S, 64:128], 0.0)
        nc.vector.memset(vk_st[:, :, KT_S:2 * KT_S, 64:65], 1.0)

        def body():
            qkvt = {}   # batch -> [q, k, v] tile views [128, S]
            if "t" in parts:
                excons = const.tile([1, 8], bf16, name="excons")
            if "m" in parts:
                atcons = const.tile([1, 8], f32, name="atcons")
            if "q" in parts:
                ycons = const.tile([1, 8], bf16, name="ycons")

            def emit_A_chunk(n):
                b, nl = divmod(n, S // AC)
                if b not in qkvt:
                    sl = b % 2
                    qkvt[b] = [q0_st[:, sl, :], q1_st[:, sl, :],
                               k_st[:, sl, :], v_st[:, sl, :]]
                q0, q1, k_, v_ = qkvt[b]
                cs = slice(nl * AC, (nl + 1) * AC)
                xc = xp.tile([128, KT_E, AC], bf16, tag="xc")
                nc.sync.dma_start(out=xc, in_=xT_r[:, :, n * AC:(n + 1) * AC])
                for m in range(3):
                    ps = pssc.tile([128, AC], f32, tag="sc")
                    for kt in range(KT_E):
                        for u in range(AC // 512):
                            nc.tensor.matmul(
                                ps[:, u * 512:(u + 1) * 512],
                                lhsT=wq_sb[:, kt, m * 128:(m + 1) * 128],
                                rhs=xc[:, kt, u * 512:(u + 1) * 512],
                                start=(kt == 0), stop=(kt == KT_E - 1))
                    if m == 0:
                        # live halves only; zero halves are persistent
                        nc.vector.tensor_scalar_add(
                            q0[0:64, cs], ps[0:64, :], bq_sb[0:64, 0:1])
                        nc.vector.tensor_scalar_add(
                            q1[64:128, cs], ps[64:128, :], bq_sb[64:128, 0:1])
                    else:
                        nc.vector.tensor_scalar_add(
                            qkvt[b][m + 1][:, cs], ps, bq_sb[:, m:m + 1])

            def emit_vt(b, kt, vk):
                """One full 128x128 transpose covers both heads' v."""
                sl = b % 2
                vt = pssc.tile([128, 128], bf16, tag="sc")
                nc.tensor.transpose(
                    vt, in_=qkvt[b][3][:, kt * 128:(kt + 1) * 128],
                    identity=id_sb)
                for h in range(HPC):
                    j = h * KT_S + kt
                    nc.vector.tensor_copy(
                        vk_st[:, sl, j, (1 - h) * 64:(2 - h) * 64],
                        vt[:, h * 64:(h + 1) * 64])
                    vk[(h, kt)] = vk_st[:, sl, j, :]

            def emit_attn_group(b, c, vk, ab, fill=None):
                """Both heads for sq chunk c (1024 wide). Score matmuls are
                K=128 with zero-padded q halves. fill: deque of closures
                (out-projection o-tile halves whose PSUM->SBUF move runs on
                ScalarE) popped one per 2 kt — their ACT copy slots into
                the exp stream so ACT never idles and the pssc slot frees
                without touching the DVE queue."""
                skip_at = "t" in parts
                cq = c * 1024
                q0, q1, k_, v_ = qkvt[b]
                qz = [q0, q1]
                at = [] if skip_at else [
                    psat.tile([128, 1024], f32, tag="at", name=f"at{b}{c}{h}")
                    for h in range(HPC)]

                def emit_at(kt, h, ex):
                    for u in range(2):
                        nc.tensor.matmul(
                            at[h][:, u * 512:(u + 1) * 512],
                            lhsT=vk[(h, kt)],
                            rhs=ex[:, u * 512:(u + 1) * 512],
                            start=(kt == 0), stop=(kt == KT_S - 1))

                pending = []
                for kt in range(KT_S):
                    ks = slice(kt * 128, (kt + 1) * 128)
                    sc = [None, None]
                    for h in range(HPC):
                        sc[h] = pssc.tile([128, 1024], f32, tag="sc", name=f"sch{h}")
                        for u in range(2):
                            nc.tensor.matmul(
                                sc[h][:, u * 512:(u + 1) * 512],
                                lhsT=k_[:, ks],
                                rhs=qz[h][:, cq + u * 512:cq + (u + 1) * 512],
                                start=True, stop=True)
                    exs = []
                    for h in range(HPC):
                        ex = expp.tile([128, 1024], bf16, tag="exp")
                        nc.scalar.activation(ex, sc[h], EXP, scale=0.125)
                        exs.append(ex)
                    if skip_at:
                        for ex in exs:
                            nc.vector.tensor_copy(excons, ex[0:1, 0:8])
                        continue
                    # flush at-matmuls two kt behind the exp stream: if PE
                    # had to wait on an exp result here it would also delay
                    # the following score matmuls and starve ACT
                    while len(pending) >= 4:
                        kp, hp, exp_ = pending.pop(0)
                        emit_at(kp, hp, exp_)
                    pending += [(kt, 0, exs[0]), (kt, 1, exs[1])]
                    if fill and kt % 2 == 1:
                        fill.popleft()()
                if skip_at:
                    return
                for kp, hp, exp_ in pending:
                    emit_at(kp, hp, exp_)
                if "m" in parts:
                    # timing-only: consume at accumulators, skip normalization
                    for h in range(HPC):
                        nc.vector.tensor_copy(atcons, at[h][0:1, 0:8])
                    return
                # normalize both heads; h's denominator row is 64h, its attn
                # rows coincide with its ab partition range (h0 -> ab upper
                # half, h1 -> lower), so the DVE multiply writes ab directly.
                cs = slice(c * 1024, (c + 1) * 1024)
                for h in range(HPC):
                    dr = h * 64
                    rs = anp.tile([65, 1024], f32, tag="norm")
                    nc.vector.reciprocal(rs[dr:dr + 1, :], at[h][dr:dr + 1, :])
                    if dr != 0:
                        nc.sync.dma_start(out=rs[0:1, :], in_=rs[dr:dr + 1, :])
                    rb = anp.tile([128, 1024], f32, tag="norm")
                    nc.gpsimd.partition_broadcast(rb, rs[0:1, :])
                    hs = slice((1 - h) * 64, (2 - h) * 64)
                    nc.vector.tensor_mul(
                        ab[hs, cs], at[h][hs, :], rb[hs, :])

            def emit_oc2(b, o, c2, yst, ab, on_act):
                """Half of one output o-tile: 2 matmuls + PSUM->SBUF move
                with bias (ScalarE when hiding in an exp stream, else DVE);
                DMA of the finished o-tile after the second half."""
                yp = pssc.tile([128, 1024], f32, tag="sc")
                for u in range(2):
                    nc.tensor.matmul(
                        yp[:, u * 512:(u + 1) * 512],
                        lhsT=wo_sb[:, o * 128:(o + 1) * 128],
                        rhs=ab[:, c2 * 1024 + u * 512:
                               c2 * 1024 + (u + 1) * 512],
                        start=True, stop=True)
                if on_act:
                    nc.scalar.activation(
                        yst[:, c2 * 1024:(c2 + 1) * 1024], yp,
                        mybir.ActivationFunctionType.Identity,
                        bias=bo_sb[:, o:o + 1])
                else:
                    nc.vector.tensor_scalar_add(
                        yst[:, c2 * 1024:(c2 + 1) * 1024], yp,
                        bo_sb[:, o:o + 1])
                if c2 == 1:
                    if "q" in parts:
                        # timing-only: consume yst without the DRAM store
                        nc.vector.tensor_copy(ycons, yst[0:1, 0:8])
                    else:
                        # alternate store queues so the 512KB transfers
                        # don't serialize behind one DMA ring
                        eng = nc.gpsimd if o % 2 == 0 else nc.scalar
                        eng.dma_start(
                            out=yT[o * 128:(o + 1) * 128, b * S:(b + 1) * S],
                            in_=yst)

            def emit_outproj_part(b, part, ab, on_act=True):
                """2 of the 8 output o-tiles for batch b, emitted directly
                (used when the fill path is off and for the tail batch)."""
                for o in (2 * part, 2 * part + 1):
                    yst = ystp.tile([128, S], bf16, tag="yst")
                    for c2 in range(2):
                        emit_oc2(b, o, c2, yst, ab, on_act)

            def outproj_fill_closures(b, ab):
                """16 closures, one per o-tile half, for interleaving into
                the next batch's attention groups."""
                cls = []
                for o in range(E // 128):
                    yst = ystp.tile([128, S], bf16, tag="yst",
                                    name=f"yst{b}{o}")
                    for c2 in range(2):
                        cls.append(
                            lambda b=b, o=o, c2=c2, yst=yst:
                            emit_oc2(b, o, c2, yst, ab, False))
                return cls

            from collections import deque
            # Interleaving outproj work into the kt loops measured worse
            # with both ACT copies (ACT is the saturated pacer in groups)
            # and DVE adds (yp slot rotation perturbs the sc tile chain);
            # boundary placement with ACT Identity copies wins.
            use_fill = False
            fill = deque()
            for n in range(S // AC):
                emit_A_chunk(n)
            abs_ = {}
            for b in range(B):
                abs_[b] = None if ("t" in parts or "m" in parts) else asp.tile(
                    [128, S], bf16, tag="ab", name=f"ab{b}")
                if "a" in parts:
                    vk = {}
                    for kt in range(KT_S):
                        emit_vt(b, kt, vk)
                if b >= 1 and use_fill:
                    fill.extend(outproj_fill_closures(b - 1, abs_[b - 1]))
                for gi in range(4):
                    if gi % 2 == 0 and "a" in parts:
                        emit_attn_group(b, gi // 2, vk, abs_[b],
                                        fill if use_fill else None)
                    if b >= 1 and "o" in parts and not use_fill:
                        emit_outproj_part(b - 1, gi, abs_[b - 1])
                    if b + 1 < B and gi % 2 == 1:
                        emit_A_chunk((S // AC) * (b + 1) + gi // 2)
                while fill:
                    fill.popleft()()
                if niter is not None and parts != "Aao" and "o" not in parts:
                    cons_b = const.tile([1, 8], bf16, name=f"cons{b}", bufs=1) \
                        if b == 0 else cons_b
                    for t in range(4):
                        nc.vector.tensor_copy(cons_b, qkvt[b][t][0:1, 0:8])
                    if "a" in parts and "t" not in parts and "m" not in parts:
                        nc.vector.tensor_copy(cons_b, abs_[b][0:1, 0:8])
            if "o" in parts:
                for gi in range(4):
                    emit_outproj_part(B - 1, gi, abs_[B - 1], on_act=False)


        if niter is None:
            body()
        else:
            with tc.For_i(0, niter, 1):
                body()
            dmy = const.tile([1, 3], f32)
            nc.vector.tensor_copy(dmy, bq_sb[0:1, 0:3])
            nc.gpsimd.dma_start(out=tout[:, :], in_=dmy)

    nc.compile()
    _prog_cache[key] = nc
    return nc


def _bf16(a):
    import ml_dtypes
    return np.ascontiguousarray(a).astype(ml_dtypes.bfloat16)


def timing_in_map():
    """Properly-dtyped random weights for the niter timing programs."""
    rng = np.random.default_rng(0)
    return {
        "wq": _bf16(rng.standard_normal((E, M3)) / 32),
        "bq": np.zeros((128, 3), np.float32),
        "wo": _bf16(rng.standard_normal((F, E)) / 32),
        "bo": np.zeros((128, E // 128), np.float32),
    }


def make_in_maps(x, W_qkv, b_qkv, W_out, b_out):
    xT = _bf16(x.reshape(BS, E).T)
    in_maps = []
    for c in range(NCORES):
        rows, brows = [], []
        for blk in range(3):
            for h in (HPC * c, HPC * c + 1):
                rows.append(W_qkv[blk * E + h * D: blk * E + (h + 1) * D, :])
                brows.append(b_qkv[blk * E + h * D: blk * E + (h + 1) * D])
        W_loc = np.concatenate(rows, axis=0)            # [384, 1024]
        b_loc = np.concatenate(brows, axis=0)           # [384]
        wq_in = _bf16(W_loc.T)
        bq_in = np.ascontiguousarray(b_loc.reshape(3, 128).T).astype(np.float32)
        # ab rows 0:64 hold head 2c+1's attn, rows 64:128 head 2c's
        wo_loc = np.concatenate(
            [W_out[:, (HPC * c + 1) * D:(HPC * c + 2) * D],
             W_out[:, HPC * c * D:(HPC * c + 1) * D]], axis=1)
        wo_in = _bf16(wo_loc.T)
        if c == 0:
            bo_in = np.ascontiguousarray(
                b_out.reshape(E // 128, 128).T).astype(np.float32)
        else:
            bo_in = np.zeros((128, E // 128), dtype=np.float32)
        in_maps.append(
            {"xT": xT, "wq": wq_in, "bq": bq_in, "wo": wo_in, "bo": bo_in})
    return in_maps


def kernel(x, W_qkv, b_qkv, W_out, b_out):
    x = np.asarray(x, dtype=np.float32)
    W_qkv = np.asarray(W_qkv, dtype=np.float32)
    b_qkv = np.asarray(b_qkv, dtype=np.float32)
    W_out = np.asarray(W_out, dtype=np.float32)
    b_out = np.asarray(b_out, dtype=np.float32)

    nc = build_program()
    in_maps = make_in_maps(x, W_qkv, b_qkv, W_out, b_out)
    res = run_bass_kernel_spmd(nc, in_maps, core_ids=list(range(NCORES)))
    acc = np.zeros((E, BS), dtype=np.float32)
    for c in range(NCORES):
        acc += res.results[c]["yT"].astype(np.float32)
    return np.ascontiguousarray(acc.T).reshape(B, S, E)


if __name__ == "__main__":
    rng = np.random.default_rng(0)
    x = rng.standard_normal((B, S, E), dtype=np.float32)
    s = 1.0 / np.sqrt(E)
    W_qkv = rng.uniform(-s, s, (3 * E, E)).astype(np.float32)
    b_qkv = rng.uniform(-s, s, (3 * E,)).astype(np.float32)
    W_out = rng.uniform(-s, s, (E, E)).astype(np.float32)
    b_out = rng.uniform(-s, s, (E,)).astype(np.float32)
    y = kernel(x, W_qkv, b_qkv, W_out, b_out)
    print("out", y.shape, y.dtype, float(np.abs(y).max()))


# revision 57
# speedup vs baseline: 1.2111x; 1.2111x over previous
"""Multi-head attention (B=4, S=2048, E=1024, H=16, D=64) on 8 TRN2 cores.

Sharding: heads 2c, 2c+1 on core c (Megatron-style column-parallel qkv,
row-parallel out-projection; bf16 partial outputs summed on host).

Per-core dataflow (bf16 operands, f32 PSUM accumulation; ~4e-3 rel err):
  A) qkvT feature-major projection of the (replicated, bf16) xT in 1024-col
     chunks; q is stored as two zero-padded copies (q0: head0 rows live,
     q1: head1 rows live) so score matmuls run K=128 with the full k tile
     stationary (HW-measured: row-tiled K=64 matmuls are ~2x slower).
  B) per (batch, sq-chunk): kt-outer loop: per head, scoresT[sk, sq]
     matmuls, exp on ScalarE (scale=1/8 folded in; |scores| < 3 so no
     max-subtraction), bf16 probs, attnT accumulation per head with [128,
     128] vk stationaries placing h0's denom at PSUM row 0 / attn at rows
     64:127 and h1's attn at rows 0:63 / denom at row 64, so the
     normalization multiplies (reciprocal + gpsimd partition-broadcast +
     DVE multiply) write bf16 ab directly, partition-aligned. at-matmuls
     are flushed two kt behind the exp stream so the PE never stalls the
     score-matmul stream (which feeds ScalarE) on an exp dependency.
  C) row-parallel out-projection of the local 128 features -> bf16 yT
     partial (summed across cores on the host); PSUM->SBUF move + bias on
     ScalarE at group boundaries (where the exp stream idles), stores
     alternate between two DMA queues.

Emission interleaves phase-A chunks of batch b+1 and out-projection parts of
batch b-1 between attention groups of batch b, so the in-order PE always has
dependency-free matmuls to fill exp/normalization stalls. ScalarE exp
(~265us busy/core) is the structural pacer; score+exp runs within ~9us of
that floor.
"""
from contextlib import ExitStack

import numpy as np

import concourse.bass as bass
import concourse.mybir as mybir
import concourse.tile as tile
from concourse import bacc
from concourse.bass_utils import run_bass_kernel_spmd
from concourse.masks import make_identity

B, S, E, H, D = 4, 2048, 1024, 16, 64
NCORES = 8
HPC = H // NCORES        # 2 heads per core
F = HPC * D              # 128 local features
M3 = 3 * F               # 384 local qkv rows
BS = B * S               # 8192
KT_E = E // 128          # 8 contraction tiles for projections
KT_S = S // 128          # 16 sk tiles
AC = 1024                # phase-A chunk width
f32 = mybir.dt.float32
bf16 = mybir.dt.bfloat16
EXP = mybir.ActivationFunctionType.Exp

_prog_cache = {}


def build_program(niter=None, parts="Aao"):
    """niter=None: normal external-I/O program. niter=N: timing variant with
    internal DRAM x/y and the whole body in a device-side For_i loop."""
    key = ("nc", niter, parts)
    if key in _prog_cache:
        return _prog_cache[key]
    nc = bacc.Bacc("TRN2", target_bir_lowering=False)
    if niter is None:
        xT = nc.dram_tensor("xT", [E, BS], bf16, kind="ExternalInput")
        yT = nc.dram_tensor("yT", [E, BS], bf16, kind="ExternalOutput")
    else:
        xT = nc.dram_tensor("xTi", [E, BS], bf16, kind="Internal")
        yT = nc.dram_tensor("yTi", [E, BS], bf16, kind="Internal")
    wq = nc.dram_tensor("wq", [E, M3], bf16, kind="ExternalInput")
    bq = nc.dram_tensor("bq", [128, 3], f32, kind="ExternalInput")
    wo = nc.dram_tensor("wo", [F, E], bf16, kind="ExternalInput")
    bo = nc.dram_tensor("bo", [128, E // 128], f32, kind="ExternalInput")
    if niter is not None:
        tout = nc.dram_tensor("tout", [1, 3], f32, kind="ExternalOutput")

    with tile.TileContext(nc) as tc, ExitStack() as ctx:
        const = ctx.enter_context(tc.tile_pool(name="const", bufs=1))
        xp = ctx.enter_context(tc.tile_pool(name="xp", bufs=2))
        expp = ctx.enter_context(tc.tile_pool(name="expp", bufs=8))
        anp = ctx.enter_context(tc.tile_pool(name="anp", bufs=4))
        asp = ctx.enter_context(tc.tile_pool(name="asp", bufs=2))
        ystp = ctx.enter_context(tc.tile_pool(name="ystp", bufs=8))
        # PSUM: "sc" slots [128,1024] f32 (scores / qkv / out-proj /
        # transposes), "at" slots [65,1024] f32 x2 heads.
        pssc = ctx.enter_context(tc.tile_pool(name="pssc", bufs=2, space="PSUM"))
        psat = ctx.enter_context(tc.tile_pool(name="psat", bufs=2, space="PSUM"))

        wq_sb = const.tile([128, KT_E, M3], bf16)
        nc.gpsimd.dma_start(out=wq_sb, in_=wq.rearrange("(kt p) m -> p kt m", p=128))
        wo_sb = const.tile([F, E], bf16)
        nc.gpsimd.dma_start(out=wo_sb, in_=wo[:, :])
        bq_sb = const.tile([128, 3], f32)
        nc.gpsimd.dma_start(out=bq_sb, in_=bq[:, :])
        bo_sb = const.tile([128, E // 128], f32)
        nc.gpsimd.dma_start(out=bo_sb, in_=bo[:, :])
        id_f32 = const.tile([128, 128], f32)
        make_identity(nc, id_f32)
        id_sb = const.tile([128, 128], bf16)
        nc.vector.tensor_copy(id_sb, id_f32)

        xT_r = xT.rearrange("(kt p) n -> p kt n", p=128)

        # persistent double-buffered qkv storage: [128, 2, S] per tensor,
        # slot b%2. q is written as two zero-padded copies (q0: head0 rows
        # live, head1 rows zero; q1 vice versa) so score matmuls run K=128
        # with the full k tile stationary — HW-measured K=64 row-tiled
        # matmuls and per-matmul weight reloads are slower. vk ones columns
        # and q zero halves are written once.
        q0_st = const.tile([128, 2, S], bf16, name="q0_st")
        q1_st = const.tile([128, 2, S], bf16, name="q1_st")
        k_st = const.tile([128, 2, S], bf16, name="k_st")
        v_st = const.tile([128, 2, S], bf16, name="v_st")
        # vk stationaries are [128, 128] so each head's attnT lands in its
        # own PSUM partition range: h0 = [ones | 0... | v0] -> denom row 0 +
        # attn rows 64:127 (-> ab[64:128]); h1 = [v1 | ones | 0...] -> attn
        # rows 0:63 (-> ab[0:64]) + denom row 64. Normalization multiplies
        # write ab directly on DVE (partition-aligned, no copy DMA); only
        # h1 needs the denominator row-shift DMA (64 -> 0) before the
        # gpsimd partition broadcast. Host reorders W_out columns to match.
        vk_st = const.tile([128, 2, HPC * KT_S, 128], bf16, name="vk_st")
        nc.vector.memset(q0_st[64:128, :, :], 0.0)
        nc.vector.memset(q1_st[0:64, :, :], 0.0)
        nc.vector.memset(vk_st[:, :, 0:KT_S, 0:64], 0.0)
        nc.vector.memset(vk_st[:, :, 0:KT_S, 0:1], 1.0)
        nc.vector.memset(vk_st[:, :, KT_S:2 * KT_S, 64:128], 0.0)
        nc.vector.memset(vk_st[:, :, KT_S:2 * KT_S, 64:65], 1.0)

        def body():
            qkvt = {}   # batch -> [q, k, v] tile views [128, S]
            if "t" in parts:
                excons = const.tile([1, 8], bf16, name="excons")
            if "m" in parts:
                atcons = const.tile([1, 8], f32, name="atcons")
            if "q" in parts:
                ycons = const.tile([1, 8], bf16, name="ycons")

            def emit_A_chunk(n):
                b, nl = divmod(n, S // AC)
                if b not in qkvt:
                    sl = b % 2
                    qkvt[b] = [q0_st[:, sl, :], q1_st[:, sl, :],
                               k_st[:, sl, :], v_st[:, sl, :]]
                q0, q1, k_, v_ = qkvt[b]
                cs = slice(nl * AC, (nl + 1) * AC)
                xc = xp.tile([128, KT_E, AC], bf16, tag="xc")
                nc.sync.dma_start(out=xc, in_=xT_r[:, :, n * AC:(n + 1) * AC])
                for m in range(3):
                    ps = pssc.tile([128, AC], f32, tag="sc")
                    for kt in range(KT_E):
                        for u in range(AC // 512):
                            nc.tensor.matmul(
                                ps[:, u * 512:(u + 1) * 512],
                                lhsT=wq_sb[:, kt, m * 128:(m + 1) * 128],
                                rhs=xc[:, kt, u * 512:(u + 1) * 512],
                                start=(kt == 0), stop=(kt == KT_E - 1))
                    if m == 0:
                        # live halves only; zero halves are persistent
                        nc.vector.tensor_scalar_add(
                            q0[0:64, cs], ps[0:64, :], bq_sb[0:64, 0:1])
                        nc.vector.tensor_scalar_add(
                            q1[64:128, cs], ps[64:128, :], bq_sb[64:128, 0:1])
                    else:
                        nc.vector.tensor_scalar_add(
                            qkvt[b][m + 1][:, cs], ps, bq_sb[:, m:m + 1])
                if "a" in parts:
                    for kt in range(nl * KT_S // 2, (nl + 1) * KT_S // 2):
                        emit_vt(b, kt)

            def emit_vt(b, kt):
                """Per-head v transposes via the DMA XBAR unit, issued
                right after the producing A-chunk (one batch early) so
                transfer latency hides behind batch b-1's groups."""
                sl = b % 2
                for h in range(HPC):
                    j = h * KT_S + kt
                    nc.sync.dma_start_transpose(
                        out=vk_st[:, sl, j, (1 - h) * 64:(2 - h) * 64],
                        in_=qkvt[b][3][h * 64:(h + 1) * 64,
                                       kt * 128:(kt + 1) * 128])

            def emit_attn_group(b, c, vk, ab, fill=None):
                """Both heads for sq chunk c (1024 wide). Score matmuls are
                K=128 with zero-padded q halves. fill: deque of closures
                (out-projection o-tile halves whose PSUM->SBUF move runs on
                ScalarE) popped one per 2 kt — their ACT copy slots into
                the exp stream so ACT never idles and the pssc slot frees
                without touching the DVE queue."""
                skip_at = "t" in parts
                cq = c * 1024
                q0, q1, k_, v_ = qkvt[b]
                qz = [q0, q1]
                at = [] if skip_at else [
                    psat.tile([128, 1024], f32, tag="at", name=f"at{b}{c}{h}")
                    for h in range(HPC)]

                def emit_at(kt, h, ex):
                    for u in range(2):
                        nc.tensor.matmul(
                            at[h][:, u * 512:(u + 1) * 512],
                            lhsT=vk[(h, kt)],
                            rhs=ex[:, u * 512:(u + 1) * 512],
                            start=(kt == 0), stop=(kt == KT_S - 1))

                pending = []
                for kt in range(KT_S):
                    ks = slice(kt * 128, (kt + 1) * 128)
                    sc = [None, None]
                    for h in range(HPC):
                        sc[h] = pssc.tile([128, 1024], f32, tag="sc", name=f"sch{h}")
                        for u in range(2):
                            nc.tensor.matmul(
                                sc[h][:, u * 512:(u + 1) * 512],
                                lhsT=k_[:, ks],
                                rhs=qz[h][:, cq + u * 512:cq + (u + 1) * 512],
                                start=True, stop=True)
                    exs = []
                    for h in range(HPC):
                        ex = expp.tile([128, 1024], bf16, tag="exp")
                        nc.scalar.activation(ex, sc[h], EXP, scale=0.125)
                        exs.append(ex)
                    if skip_at:
                        for ex in exs:
                            nc.vector.tensor_copy(excons, ex[0:1, 0:8])
                        continue
                    # flush at-matmuls two kt behind the exp stream: if PE
                    # had to wait on an exp result here it would also delay
                    # the following score matmuls and starve ACT
                    while len(pending) >= 4:
                        kp, hp, exp_ = pending.pop(0)
                        emit_at(kp, hp, exp_)
                    pending += [(kt, 0, exs[0]), (kt, 1, exs[1])]
                    if fill and kt % 2 == 1:
                        fill.popleft()()
                if skip_at:
                    return
                for kp, hp, exp_ in pending:
                    emit_at(kp, hp, exp_)
                if "m" in parts:
                    # timing-only: consume at accumulators, skip normalization
                    for h in range(HPC):
                        nc.vector.tensor_copy(atcons, at[h][0:1, 0:8])
                    return
                # normalize both heads; h's denominator row is 64h, its attn
                # rows coincide with its ab partition range (h0 -> ab upper
                # half, h1 -> lower), so the DVE multiply writes ab directly.
                cs = slice(c * 1024, (c + 1) * 1024)
                for h in range(HPC):
                    dr = h * 64
                    rs = anp.tile([65, 1024], f32, tag="norm")
                    nc.vector.reciprocal(rs[dr:dr + 1, :], at[h][dr:dr + 1, :])
                    if dr != 0:
                        nc.sync.dma_start(out=rs[0:1, :], in_=rs[dr:dr + 1, :])
                    rb = anp.tile([128, 1024], f32, tag="norm")
                    nc.gpsimd.partition_broadcast(rb, rs[0:1, :])
                    hs = slice((1 - h) * 64, (2 - h) * 64)
                    nc.vector.tensor_mul(
                        ab[hs, cs], at[h][hs, :], rb[hs, :])

            def emit_oc2(b, o, c2, yst, ab, on_act):
                """Half of one output o-tile: 2 matmuls + PSUM->SBUF move
                with bias (ScalarE when hiding in an exp stream, else DVE);
                DMA of the finished o-tile after the second half."""
                yp = pssc.tile([128, 1024], f32, tag="sc")
                for u in range(2):
                    nc.tensor.matmul(
                        yp[:, u * 512:(u + 1) * 512],
                        lhsT=wo_sb[:, o * 128:(o + 1) * 128],
                        rhs=ab[:, c2 * 1024 + u * 512:
                               c2 * 1024 + (u + 1) * 512],
                        start=True, stop=True)
                if on_act:
                    nc.scalar.activation(
                        yst[:, c2 * 1024:(c2 + 1) * 1024], yp,
                        mybir.ActivationFunctionType.Identity,
                        bias=bo_sb[:, o:o + 1])
                else:
                    nc.vector.tensor_scalar_add(
                        yst[:, c2 * 1024:(c2 + 1) * 1024], yp,
                        bo_sb[:, o:o + 1])
                if c2 == 1:
                    if "q" in parts:
                        # timing-only: consume yst without the DRAM store
                        nc.vector.tensor_copy(ycons, yst[0:1, 0:8])
                    else:
                        # alternate store queues so the 512KB transfers
                        # don't serialize behind one DMA ring
                        eng = nc.gpsimd if o % 2 == 0 else nc.scalar
                        eng.dma_start(
                            out=yT[o * 128:(o + 1) * 128, b * S:(b + 1) * S],
                            in_=yst)

            def emit_outproj_part(b, part, ab, on_act=True):
                """2 of the 8 output o-tiles for batch b, emitted directly
                (used when the fill path is off and for the tail batch)."""
                for o in (2 * part, 2 * part + 1):
                    yst = ystp.tile([128, S], bf16, tag="yst")
                    for c2 in range(2):
                        emit_oc2(b, o, c2, yst, ab, on_act)

            def outproj_fill_closures(b, ab):
                """16 closures, one per o-tile half, for interleaving into
                the next batch's attention groups."""
                cls = []
                for o in range(E // 128):
                    yst = ystp.tile([128, S], bf16, tag="yst",
                                    name=f"yst{b}{o}")
                    for c2 in range(2):
                        cls.append(
                            lambda b=b, o=o, c2=c2, yst=yst:
                            emit_oc2(b, o, c2, yst, ab, False))
                return cls

            from collections import deque
            # Interleaving outproj work into the kt loops measured worse
            # with both ACT copies (ACT is the saturated pacer in groups)
            # and DVE adds (yp slot rotation perturbs the sc tile chain);
            # boundary placement with ACT Identity copies wins.
            use_fill = False
            fill = deque()
            for n in range(S // AC):
                emit_A_chunk(n)
            abs_ = {}
            for b in range(B):
                abs_[b] = None if ("t" in parts or "m" in parts) else asp.tile(
                    [128, S], bf16, tag="ab", name=f"ab{b}")
                if "a" in parts:
                    vk = {(h, kt): vk_st[:, b % 2, h * KT_S + kt, :]
                          for h in range(HPC) for kt in range(KT_S)}
                if b >= 1 and use_fill:
                    fill.extend(outproj_fill_closures(b - 1, abs_[b - 1]))
                for gi in range(4):
                    if gi % 2 == 0 and "a" in parts:
                        emit_attn_group(b, gi // 2, vk, abs_[b],
                                        fill if use_fill else None)
                    if b >= 1 and "o" in parts and not use_fill:
                        emit_outproj_part(b - 1, gi, abs_[b - 1])
                    if b + 1 < B and gi % 2 == 1:
                        emit_A_chunk((S // AC) * (b + 1) + gi // 2)
                while fill:
                    fill.popleft()()
                if niter is not None and parts != "Aao" and "o" not in parts:
                    cons_b = const.tile([1, 8], bf16, name=f"cons{b}", bufs=1) \
                        if b == 0 else cons_b
                    for t in range(4):
                        nc.vector.tensor_copy(cons_b, qkvt[b][t][0:1, 0:8])
                    if "a" in parts and "t" not in parts and "m" not in parts:
                        nc.vector.tensor_copy(cons_b, abs_[b][0:1, 0:8])
            if "o" in parts:
                for gi in range(4):
                    emit_outproj_part(B - 1, gi, abs_[B - 1], on_act=False)


        if niter is None:
            body()
        else:
            with tc.For_i(0, niter, 1):
                body()
            dmy = const.tile([1, 3], f32)
            nc.vector.tensor_copy(dmy, bq_sb[0:1, 0:3])
            nc.gpsimd.dma_start(out=tout[:, :], in_=dmy)

    nc.compile()
    _prog_cache[key] = nc
    return nc


def _bf16(a):
    import ml_dtypes
    return np.ascontiguousarray(a).astype(ml_dtypes.bfloat16)


def timing_in_map():
    """Properly-dtyped random weights for the niter timing programs."""
    rng = np.random.default_rng(0)
    return {
        "wq": _bf16(rng.standard_normal((E, M3)) / 32),
        "bq": np.zeros((128, 3), np.float32),
        "wo": _bf16(rng.standard_normal((F, E)) / 32),
        "bo": np.zeros((128, E // 128), np.float32),
    }


def make_in_maps(x, W_qkv, b_qkv, W_out, b_out):
    xT = _bf16(x.reshape(BS, E).T)
    in_maps = []
    for c in range(NCORES):
        rows, brows = [], []
        for blk in range(3):
            for h in (HPC * c, HPC * c + 1):
                rows.append(W_qkv[blk * E + h * D: blk * E + (h + 1) * D, :])
                brows.append(b_qkv[blk * E + h * D: blk * E + (h + 1) * D])
        W_loc = np.concatenate(rows, axis=0)            # [384, 1024]
        b_loc = np.concatenate(brows, axis=0)           # [384]
        wq_in = _bf16(W_loc.T)
        bq_in = np.ascontiguousarray(b_loc.reshape(3, 128).T).astype(np.float32)
        # ab rows 0:64 hold head 2c+1's attn, rows 64:128 head 2c's
        wo_loc = np.concatenate(
            [W_out[:, (HPC * c + 1) * D:(HPC * c + 2) * D],
             W_out[:, HPC * c * D:(HPC * c + 1) * D]], axis=1)
        wo_in = _bf16(wo_loc.T)
        if c == 0:
            bo_in = np.ascontiguousarray(
                b_out.reshape(E // 128, 128).T).astype(np.float32)
        else:
            bo_in = np.zeros((128, E // 128), dtype=np.float32)
        in_maps.append(
            {"xT": xT, "wq": wq_in, "bq": bq_in, "wo": wo_in, "bo": bo_in})
    return in_maps


def kernel(x, W_qkv, b_qkv, W_out, b_out):
    x = np.asarray(x, dtype=np.float32)
    W_qkv = np.asarray(W_qkv, dtype=np.float32)
    b_qkv = np.asarray(b_qkv, dtype=np.float32)
    W_out = np.asarray(W_out, dtype=np.float32)
    b_out = np.asarray(b_out, dtype=np.float32)

    nc = build_program()
    in_maps = make_in_maps(x, W_qkv, b_qkv, W_out, b_out)
    res = run_bass_kernel_spmd(nc, in_maps, core_ids=list(range(NCORES)))
    acc = np.zeros((E, BS), dtype=np.float32)
    for c in range(NCORES):
        acc += res.results[c]["yT"].astype(np.float32)
    return np.ascontiguousarray(acc.T).reshape(B, S, E)


if __name__ == "__main__":
    rng = np.random.default_rng(0)
    x = rng.standard_normal((B, S, E), dtype=np.float32)
    s = 1.0 / np.sqrt(E)
    W_qkv = rng.uniform(-s, s, (3 * E, E)).astype(np.float32)
    b_qkv = rng.uniform(-s, s, (3 * E,)).astype(np.float32)
    W_out = rng.uniform(-s, s, (E, E)).astype(np.float32)
    b_out = rng.uniform(-s, s, (E,)).astype(np.float32)
    y = kernel(x, W_qkv, b_qkv, W_out, b_out)
    print("out", y.shape, y.dtype, float(np.abs(y).max()))
